# revision 31
# baseline (speedup 1.0000x reference)
"""Antialiased bicubic 4x downscale (blur -> bicubic/2, twice) on 8 TRN2 cores.

The whole chain is linear and separable: every stage is M_H (x) M_W acting on
the H/W axes, so the composition collapses to a single 1024->256 banded matrix
T applied on both sides: out = T @ X @ T^T per (batch, channel) image.

Sharding: pure data parallel - batch 16 -> 2 per core, 6 images/core.

The kernel is HBM-bound: 24 MiB of x per core streams at line rate (~360
GB/s, the 8-core fair share of the chip HBM ceiling) on the sync HWDGE ring
-- HWDGE starts ~1.3 us earlier than SWDGE and is immune to the DVE 2-port
perf-mode lockout that starves SWDGE descriptor generation.  Mid-stream z
stores ride the scalar HWDGE ring; the last image's stores go back to the
(by then idle) sync engine so store issues never sit in ACT's evac FIFO.
Descriptors are kept >= 2 KB, and regions being DMA-written are never read
concurrently (both measurably collapse the drain rate).  All compute
(pass 1 f32r matmuls exploiting T's band sparsity, PE transposes, bf16
pass 2) hides under the stream; work after the last byte is minimized:

  The last image arrives as ch0 (cols 0-511, processed classically),
  then cols 512-1023 in three row-block groups, cast to bf16 on-chip.
  Those columns take a transposed pass 1: Yt[qc 4-7] accumulate with the
  bf16 x row-blocks as stationary against banded Tt windows, into three
  PSUM region tiles split at the pc4/pc6 window starts (cols 126/190) so
  each region evacuates as soon as its last contributing row-block is in.
  z is split at column 126: z[:, :, 0:126] (qc 0-3 only) stores early;
  z[:, 0, 126:256] (needs Yt cols 0:128 = regions A+B) computes and
  stores while the final chunks stream; after the last byte only the
  pc6/7 windowed matmuls, one region evac, 4 pass-2 matmuls, one evac
  and one 130-column store remain.
"""

import numpy as np
import ml_dtypes

import concourse.bacc as bacc
import concourse.mybir as mybir
import concourse.tile as tile
from concourse.bass_utils import run_bass_kernel_spmd

SIGMA = 0.66
BICUBIC_W = np.array([-0.09375, 0.59375, 0.59375, -0.09375], dtype=np.float64)

N_CORES = 8
B, C, H, W = 16, 3, 1024, 1024
HO = H // 4
IMGS = (B // N_CORES) * C  # 6 images per core

F32 = mybir.dt.float32
F32R = mybir.dt.float32r
BF16 = mybir.dt.bfloat16

# z column split for the last image: z[:, 0:ZCUT] depends only on qc 0-3
# (x cols 0-511) and is stored early; z[:, ZCUT:] additionally gets the
# qc 4-7 contributions added after the stream ends.
ZCUT = 126


def _gauss_matrix(n):
    x = np.arange(3, dtype=np.float32) - np.float32(1.0)
    k = np.exp(np.float32(-0.5) * (x / np.float32(SIGMA)) ** 2)
    k = (k / k.sum()).astype(np.float64)
    G = np.zeros((n, n))
    for t in range(3):
        G += k[t] * np.eye(n, n, t - 1)
    return G


def _down_matrix(n):
    # out[i] = sum_t w[t] * x[clamp(2i + t - 1, 0, n-1)]
    m = n // 2
    D = np.zeros((m, n))
    for i in range(m):
        for t in range(4):
            j = min(max(2 * i + t - 1, 0), n - 1)
            D[i, j] += BICUBIC_W[t]
    return D


def build_T():
    T = _down_matrix(H // 2) @ _gauss_matrix(H // 2) @ _down_matrix(H) @ _gauss_matrix(H)
    return T.astype(np.float32)  # [256, 1024]


def _pass1_pieces(Tt):
    """(pc, ih) pairs where Tt[128pc:128pc+128, 128ih:128ih+128] is nonzero."""
    pieces = []
    for ih in range(2):
        for pc in range(8):
            if np.any(Tt[128 * pc : 128 * (pc + 1), 128 * ih : 128 * (ih + 1)]):
                pieces.append((pc, ih))
    return pieces


def _pc_windows(Tt):
    """Per row-block pc, the [a, b) span of nonzero columns of Tt."""
    wins = []
    for pc in range(8):
        nz = np.nonzero(np.any(Tt[128 * pc : 128 * (pc + 1), :] != 0, axis=0))[0]
        wins.append((int(nz.min()), int(nz.max()) + 1))
    return wins


def _build_graph():
    Tt = build_T().T  # [1024, 256]
    pieces = _pass1_pieces(Tt)
    pcs_by_ih = [[pc for (pc, ih2) in pieces if ih2 == ih] for ih in range(2)]
    wins = _pc_windows(Tt)

    nc = bacc.Bacc("TRN2", target_bir_lowering=False, debug=False)
    x = nc.dram_tensor("x", [IMGS, H, W], F32R, kind="ExternalInput").ap()
    # tb is host-prearranged to the SBUF layout: tb[p, c, n] = Tt[128c+p, n]
    tb = nc.dram_tensor("tb", [128, 8, HO], BF16, kind="ExternalInput").ap()
    eye = nc.dram_tensor("eye", [128, 128], BF16, kind="ExternalInput").ap()
    # out in SBUF layout [p, img, c, j] = Z[img, 128c+p, j]; host unscrambles
    out = nc.dram_tensor("out", [128, IMGS, 2, HO], F32, kind="ExternalOutput").ap()

    with tile.TileContext(nc) as tc:
        with (
            tc.tile_pool(name="const", bufs=1) as cpool,
            tc.tile_pool(name="xin", bufs=4) as xpool,
            tc.tile_pool(name="xbin", bufs=1) as xbpool,
            tc.tile_pool(name="ysb", bufs=2) as ypool,
            tc.tile_pool(name="ytsb", bufs=2) as ytpool,
            tc.tile_pool(name="zout", bufs=2) as zpool,
            tc.tile_pool(name="psy", bufs=4, space="PSUM") as psy,
            tc.tile_pool(name="pst", bufs=2, space="PSUM") as pst,
            tc.tile_pool(name="ps2", bufs=2, space="PSUM") as ps2,
        ):
            # tiny warmup load: spins up the SWDGE queue/engines so the real
            # stream's first bytes land sooner
            warm = cpool.tile([128, 8], F32R, tag="warm")
            nc.sync.dma_start(out=warm[:], in_=x[0, 0:128, 0:8])

            ttb = cpool.tile([128, 8, HO], BF16, tag="ttb")
            nc.scalar.dma_start(out=ttb[:], in_=tb)
            ident = cpool.tile([128, 128], BF16, tag="ident")
            nc.scalar.dma_start(out=ident[:], in_=eye)
            # f32r copy of Tt for pass 1 stationary, cast on-chip
            tt = cpool.tile([128, 8, HO], F32R, tag="tt")
            nc.vector.tensor_copy(tt[:], ttb[:])

            def p1mm(yq, pc, ih, xap, start, stop):
                nc.tensor.matmul(
                    yq,
                    tt[:, pc, 128 * ih : 128 * (ih + 1)],
                    xap,
                    start=start,
                    stop=stop,
                )

            for img in range(IMGS):
                xt = xpool.tile([128, 8, W], F32R, tag="xt", name=f"xt{img}")
                xr = x[img].rearrange("(c p) w -> p c w", p=128)

                y_sb = ypool.tile([128, 2, W], BF16)
                yt_sb = ytpool.tile([128, 8, HO], BF16)
                z = zpool.tile([128, 2, HO], F32, tag="zout", name=f"z{img}")

                def evac(dst, src, ih):
                    if ih == 0:
                        nc.vector.tensor_copy(dst, src)
                    else:
                        nc.scalar.copy(dst, src)

                def transposes(ih, qc0, nqc, tag):
                    tp = pst.tile(
                        [128, 512], BF16, tag="pst",
                        name=f"tp{img}_{tag}_{ih}",
                    )
                    for s in range(nqc):
                        qc = qc0 + s
                        nc.tensor.matmul(
                            tp[:, 128 * s : 128 * (s + 1)],
                            y_sb[:, ih, 128 * qc : 128 * (qc + 1)],
                            ident[:],
                            is_transpose=True,
                            start=(s == 0),
                            stop=(s == nqc - 1),
                        )
                    dst = yt_sb[:, qc0 : qc0 + nqc, 128 * ih : 128 * (ih + 1)]
                    tsrc = tp[:, 0 : 128 * nqc].rearrange("p (s w) -> p s w", s=nqc)
                    evac(dst, tsrc, ih)

                def p2mm(acc, qc, ih, jslice, start, stop):
                    nc.tensor.matmul(
                        acc,
                        yt_sb[:, qc, 128 * ih : 128 * (ih + 1)],
                        ttb[:, qc, jslice],
                        start=start,
                        stop=stop,
                    )

                if img < IMGS - 1:
                    # row-block chunked loads; 4 KB descriptors
                    nc.sync.dma_start(out=xt[:, 0:4], in_=xr[:, 0:4])
                    nc.sync.dma_start(out=xt[:, 4:8], in_=xr[:, 4:8])
                    for ch in range(2):
                        for ih in range(2):
                            yq = psy.tile(
                                [128, 512], F32, tag="psy",
                                name=f"psy{img}_{ch}_{ih}",
                            )
                            pcs = pcs_by_ih[ih]
                            for k, pc in enumerate(pcs):
                                p1mm(yq[:], pc, ih,
                                     xt[:, pc, 512 * ch : 512 * (ch + 1)],
                                     k == 0, k == len(pcs) - 1)
                            evac(y_sb[:, ih, 512 * ch : 512 * (ch + 1)], yq[:], ih)
                        for ih in range(2):
                            transposes(ih, 4 * ch, 4, f"c{ch}")
                    for ih in range(2):
                        acc = ps2.tile([128, HO], F32, tag="ps2",
                                       name=f"ps2_{img}_{ih}")
                        for qc in range(8):
                            p2mm(acc[:], qc, ih, slice(0, HO), qc == 0, qc == 7)
                        evac(z[:, ih, :], acc[:], ih)
                    nc.scalar.dma_start(out=out[:, img], in_=z[:])
                else:
                    # last image: ch0 (cols 0-511, f32r), then cols 512-1023
                    # in three row-block groups cast to bf16 in-flight
                    # (2 KB source descriptors throughout)
                    xb = xbpool.tile([128, 8, 512], BF16, tag="xb")
                    nc.sync.dma_start(out=xt[:, :, 0:512], in_=xr[:, :, 0:512])
                    nc.sync.dma_start(out=xt[:, 0:4, 512:1024],
                                        in_=xr[:, 0:4, 512:1024])
                    nc.sync.dma_start(out=xt[:, 4:6, 512:1024],
                                        in_=xr[:, 4:6, 512:1024])
                    nc.sync.dma_start(out=xt[:, 6:8, 512:1024],
                                        in_=xr[:, 6:8, 512:1024])

                    # ch0: classic pass 1 in bf16 (the f32r moving
                    # operand runs at 2 cycles/col, so casting ch0 on both
                    # copy engines in parallel then matmul-ing in bf16
                    # shortens the post-arrival serial chain by ~1 us) +
                    # transposes -> yt qc 0-3
                    xb0 = xbpool.tile([128, 8, 512], BF16, tag="xb0")
                    nc.vector.tensor_copy(xb0[:, 0:4], xt[:, 0:4, 0:512])
                    nc.scalar.copy(xb0[:, 4:8], xt[:, 4:8, 0:512])
                    for ih in range(2):
                        yq = psy.tile([128, 512], F32, tag="psy",
                                      name=f"psyL_{ih}")
                        pcs = pcs_by_ih[ih]
                        for k, pc in enumerate(pcs):
                            nc.tensor.matmul(
                                yq[:],
                                ttb[:, pc, 128 * ih : 128 * (ih + 1)],
                                xb0[:, pc, :],
                                start=(k == 0),
                                stop=(k == len(pcs) - 1),
                            )
                        evac(y_sb[:, ih, 0:512], yq[:], ih)
                    for ih in range(2):
                        transposes(ih, 0, 4, "L")

                    # ch1 row-groups are cast to bf16 as they arrive (the
                    # in-flight SWDGE cast drains erratically, so the x
                    # stream stays f32r).  DVE/ACT are strict FIFO, so each
                    # cast is emitted at its pipeline position: a cast that
                    # waits on a late chunk must not precede early evacs.

                    # cols 512-1023 via transposed pass 1: Yt[qc 4-7]
                    # accumulated with bf16 x row-blocks as stationary (FWL)
                    # against banded Tt windows, into three PSUM region
                    # tiles split at the pc4/pc6 window starts so each
                    # region's accumulation closes as soon as its last
                    # contributing row-block arrives (A <- pc3, B <- pc5,
                    # C <- pc7) and is evacuated immediately.
                    SA, SB = wins[4][0], wins[6][0]
                    ytqA = psy.tile([128, 4, SA], F32, tag="psy", name="ytqA")
                    ytqB = psy.tile([128, 4, SB - SA], F32, tag="psy",
                                    name="ytqB")
                    ytqC = psy.tile([128, 4, HO - SB], F32, tag="psy",
                                    name="ytqC")
                    regions = [(0, SA, ytqA), (SA, SB, ytqB), (SB, HO, ytqC)]

                    sched = []
                    for pc in range(8):
                        a, b = wins[pc]
                        for qc in (4, 5, 6, 7):
                            for ti, (ra, rb, _t) in enumerate(regions):
                                sa, sb = max(a, ra), min(b, rb)
                                if sa < sb:
                                    sched.append((pc, qc, ti, sa, sb))
                    first_w = {}
                    last_w = {}
                    for w in sched:
                        first_w.setdefault(w[2], w)
                        last_w[w[2]] = w

                    def p1t(pcg):
                        for pc in pcg:
                            a, b = wins[pc]
                            for qc in (4, 5, 6, 7):
                                for ti, (ra, rb, t) in enumerate(regions):
                                    sa, sb = max(a, ra), min(b, rb)
                                    if sa >= sb:
                                        continue
                                    w = (pc, qc, ti, sa, sb)
                                    nc.tensor.matmul(
                                        t[:, qc - 4, sa - ra : sb - ra],
                                        xb[:, pc,
                                           128 * (qc - 4) : 128 * (qc - 3)],
                                        ttb[:, pc, sa:sb],
                                        start=(w == first_w[ti]),
                                        stop=(w == last_w[ti]),
                                    )

                    # zA: narrow qc 0-3 block for cols 0-125, computed in
                    # the PE idle gap after ch0 (before the ch1 chunks land)
                    # and stored early
                    for ih in range(2):
                        acc = ps2.tile([128, ZCUT], F32, tag="ps2",
                                       name=f"ps2A_{ih}")
                        for qc in range(4):
                            p2mm(acc[:], qc, ih, slice(0, ZCUT),
                                 qc == 0, qc == 3)
                        evac(z[:, ih, 0:ZCUT], acc[:], ih)
                    nc.sync.dma_start(out=out[:, img, :, 0:ZCUT],
                                       in_=z[:, :, 0:ZCUT])

                    # z cols 126-255 accumulate per ih in separate banks;
                    # the qc3 contribution (from ch0) starts each bank early
                    zb0 = ps2.tile([128, HO - ZCUT], F32, tag="ps2",
                                   name="zb0")
                    zb1 = ps2.tile([128, HO - ZCUT], F32, tag="ps2",
                                   name="zb1")
                    nc.tensor.matmul(zb0[:], yt_sb[:, 3, 0:128],
                                     ttb[:, 3, ZCUT:HO],
                                     start=True, stop=False)
                    nc.tensor.matmul(zb1[:], yt_sb[:, 3, 128:256],
                                     ttb[:, 3, ZCUT:HO],
                                     start=True, stop=False)

                    nc.vector.tensor_copy(xb[:, 0:4], xt[:, 0:4, 512:1024])
                    p1t((0, 1, 2, 3))
                    nc.vector.tensor_copy(yt_sb[:, 4:8, 0:SA], ytqA[:])
                    nc.scalar.copy(xb[:, 4:6], xt[:, 4:6, 512:1024])
                    p1t((4, 5))
                    nc.scalar.copy(yt_sb[:, 4:8, SA:SB], ytqB[:])

                    # z[:, 0, 126:] needs Yt cols 0:128 (regions A+B):
                    # computed and stored while pc6/pc7 still stream
                    for qc in range(4, 8):
                        nc.tensor.matmul(zb0[:], yt_sb[:, qc, 0:128],
                                         ttb[:, qc, ZCUT:HO],
                                         start=False, stop=(qc == 7))
                    nc.vector.tensor_copy(z[:, 0, ZCUT:HO], zb0[:])
                    nc.sync.dma_start(out=out[:, img, 0, ZCUT:HO],
                                       in_=z[:, 0, ZCUT:HO])

                    nc.vector.tensor_copy(xb[:, 6:7], xt[:, 6:7, 512:1024])
                    nc.scalar.copy(xb[:, 7:8], xt[:, 7:8, 512:1024])
                    p1t((6, 7))
                    nc.scalar.copy(yt_sb[:, 4:8, SB:HO], ytqC[:])
                    for qc in range(4, 8):
                        nc.tensor.matmul(zb1[:], yt_sb[:, qc, 128:256],
                                         ttb[:, qc, ZCUT:HO],
                                         start=False, stop=(qc == 7))
                    nc.vector.tensor_copy(z[:, 1, ZCUT:HO], zb1[:])
                    nc.sync.dma_start(out=out[:, img, 1, ZCUT:HO],
                                       in_=z[:, 1, ZCUT:HO])
    nc.compile()
    return nc


_GRAPH = None


def _get_graph():
    global _GRAPH
    if _GRAPH is None:
        _GRAPH = _build_graph()
    return _GRAPH


def run(x, **spmd_kwargs):
    x = np.ascontiguousarray(np.asarray(x, dtype=np.float32))
    assert x.shape == (B, C, H, W)
    nc = _get_graph()
    Tt = build_T().T  # [1024, 256] f32
    tb_host = np.ascontiguousarray(
        Tt.reshape(8, 128, HO).transpose(1, 0, 2)
    ).astype(ml_dtypes.bfloat16)
    eye_host = np.eye(128, dtype=ml_dtypes.bfloat16)
    per_core = B // N_CORES
    in_maps = [
        {
            "x": x[i * per_core : (i + 1) * per_core].reshape(IMGS, H, W),
            "tb": tb_host,
            "eye": eye_host,
        }
        for i in range(N_CORES)
    ]
    res = run_bass_kernel_spmd(nc, in_maps, core_ids=list(range(N_CORES)), **spmd_kwargs)
    outs = []
    for r in res.results:
        o = r["out"].transpose(1, 2, 0, 3).reshape(IMGS, 2 * 128, HO)
        outs.append(o.reshape(per_core, C, HO, HO))
    return np.concatenate(outs, axis=0), res


def kernel(x):
    out, _ = run(x)
    return out



# revision 32
# speedup vs baseline: 1.1320x; 1.1320x over previous
"""Antialiased bicubic 4x downscale (blur -> bicubic/2, twice) on 8 TRN2 cores.

The whole chain is linear and separable: every stage is M_H (x) M_W acting on
the H/W axes, so the composition collapses to a single 1024->256 banded matrix
T applied on both sides: out = T @ X @ T^T per (batch, channel) image.

Sharding: pure data parallel - batch 16 -> 2 per core, 6 images/core.

The kernel is HBM-bound: 24 MiB of x per core streams at line rate (~360
GB/s, the 8-core fair share of the chip HBM ceiling) on the sync HWDGE ring
-- HWDGE starts ~1.3 us earlier than SWDGE and is immune to the DVE 2-port
perf-mode lockout that starves SWDGE descriptor generation.  Mid-stream z
stores ride the scalar HWDGE ring; the last image's stores go back to the
(by then idle) sync engine so store issues never sit in ACT's evac FIFO.
Descriptors are kept >= 2 KB, and regions being DMA-written are never read
concurrently (both measurably collapse the drain rate).  All compute
(pass 1 f32r matmuls exploiting T's band sparsity, PE transposes, bf16
pass 2) hides under the stream; work after the last byte is minimized:

  The last image arrives as ch0 (cols 0-511, cast to bf16 on both copy
  engines in parallel and processed classically -- bf16 matmuls halve the
  post-arrival serial chain vs f32r),
  then cols 512-1023 in three row-block groups, cast to bf16 on-chip.
  Those columns take a transposed pass 1: Yt[qc 4-7] accumulate with the
  bf16 x row-blocks as stationary against banded Tt windows, into three
  PSUM region tiles split at the pc4/pc6 window starts (cols 126/190) so
  each region evacuates as soon as its last contributing row-block is in.
  z is split at column 126: z[:, :, 0:126] (qc 0-3 only) stores early;
  z[:, 0, 126:256] (needs Yt cols 0:128 = regions A+B) computes and
  stores while the final chunks stream; after the last byte only the
  pc6/7 windowed matmuls, one region evac, 4 pass-2 matmuls, one evac
  and one 130-column store remain.
"""

import numpy as np
import ml_dtypes

import concourse.bacc as bacc
import concourse.mybir as mybir
import concourse.tile as tile
from concourse.bass_utils import run_bass_kernel_spmd

SIGMA = 0.66
BICUBIC_W = np.array([-0.09375, 0.59375, 0.59375, -0.09375], dtype=np.float64)

N_CORES = 8
B, C, H, W = 16, 3, 1024, 1024
HO = H // 4
IMGS = (B // N_CORES) * C  # 6 images per core

F32 = mybir.dt.float32
F32R = mybir.dt.float32r
BF16 = mybir.dt.bfloat16

# z column split for the last image: z[:, 0:ZCUT] depends only on qc 0-3
# (x cols 0-511) and is stored early; z[:, ZCUT:] additionally gets the
# qc 4-7 contributions added after the stream ends.
ZCUT = 126


def _gauss_matrix(n):
    x = np.arange(3, dtype=np.float32) - np.float32(1.0)
    k = np.exp(np.float32(-0.5) * (x / np.float32(SIGMA)) ** 2)
    k = (k / k.sum()).astype(np.float64)
    G = np.zeros((n, n))
    for t in range(3):
        G += k[t] * np.eye(n, n, t - 1)
    return G


def _down_matrix(n):
    # out[i] = sum_t w[t] * x[clamp(2i + t - 1, 0, n-1)]
    m = n // 2
    D = np.zeros((m, n))
    for i in range(m):
        for t in range(4):
            j = min(max(2 * i + t - 1, 0), n - 1)
            D[i, j] += BICUBIC_W[t]
    return D


def build_T():
    T = _down_matrix(H // 2) @ _gauss_matrix(H // 2) @ _down_matrix(H) @ _gauss_matrix(H)
    return T.astype(np.float32)  # [256, 1024]


def _pass1_pieces(Tt):
    """(pc, ih) pairs where Tt[128pc:128pc+128, 128ih:128ih+128] is nonzero."""
    pieces = []
    for ih in range(2):
        for pc in range(8):
            if np.any(Tt[128 * pc : 128 * (pc + 1), 128 * ih : 128 * (ih + 1)]):
                pieces.append((pc, ih))
    return pieces


def _pc_windows(Tt):
    """Per row-block pc, the [a, b) span of nonzero columns of Tt."""
    wins = []
    for pc in range(8):
        nz = np.nonzero(np.any(Tt[128 * pc : 128 * (pc + 1), :] != 0, axis=0))[0]
        wins.append((int(nz.min()), int(nz.max()) + 1))
    return wins


def _build_graph():
    Tt = build_T().T  # [1024, 256]
    pieces = _pass1_pieces(Tt)
    pcs_by_ih = [[pc for (pc, ih2) in pieces if ih2 == ih] for ih in range(2)]
    wins = _pc_windows(Tt)

    nc = bacc.Bacc("TRN2", target_bir_lowering=False, debug=False)
    x = nc.dram_tensor("x", [IMGS, H, W], F32R, kind="ExternalInput").ap()
    # tb is host-prearranged to the SBUF layout: tb[p, c, n] = Tt[128c+p, n]
    tb = nc.dram_tensor("tb", [128, 8, HO], BF16, kind="ExternalInput").ap()
    eye = nc.dram_tensor("eye", [128, 128], BF16, kind="ExternalInput").ap()
    # out in SBUF layout [p, img, c, j] = Z[img, 128c+p, j]; host unscrambles
    out = nc.dram_tensor("out", [128, IMGS, 2, HO], F32, kind="ExternalOutput").ap()

    with tile.TileContext(nc) as tc:
        with (
            tc.tile_pool(name="const", bufs=1) as cpool,
            tc.tile_pool(name="xin", bufs=4) as xpool,
            tc.tile_pool(name="xbin", bufs=1) as xbpool,
            tc.tile_pool(name="ysb", bufs=2) as ypool,
            tc.tile_pool(name="ytsb", bufs=2) as ytpool,
            tc.tile_pool(name="zout", bufs=2) as zpool,
            tc.tile_pool(name="psy", bufs=4, space="PSUM") as psy,
            tc.tile_pool(name="pst", bufs=2, space="PSUM") as pst,
            tc.tile_pool(name="ps2", bufs=2, space="PSUM") as ps2,
        ):
            # tiny warmup load: spins up the SWDGE queue/engines so the real
            # stream's first bytes land sooner
            warm = cpool.tile([128, 8], F32R, tag="warm")
            nc.sync.dma_start(out=warm[:], in_=x[0, 0:128, 0:8])

            ttb = cpool.tile([128, 8, HO], BF16, tag="ttb")
            nc.scalar.dma_start(out=ttb[:], in_=tb)
            ident = cpool.tile([128, 128], BF16, tag="ident")
            nc.scalar.dma_start(out=ident[:], in_=eye)
            # f32r copy of Tt for pass 1 stationary, cast on-chip
            tt = cpool.tile([128, 8, HO], F32R, tag="tt")
            nc.vector.tensor_copy(tt[:], ttb[:])

            def p1mm(yq, pc, ih, xap, start, stop):
                nc.tensor.matmul(
                    yq,
                    tt[:, pc, 128 * ih : 128 * (ih + 1)],
                    xap,
                    start=start,
                    stop=stop,
                )

            for img in range(IMGS):
                xt = xpool.tile([128, 8, W], F32R, tag="xt", name=f"xt{img}")
                xr = x[img].rearrange("(c p) w -> p c w", p=128)

                y_sb = ypool.tile([128, 2, W], BF16)
                yt_sb = ytpool.tile([128, 8, HO], BF16)
                z = zpool.tile([128, 2, HO], F32, tag="zout", name=f"z{img}")

                def evac(dst, src, ih):
                    if ih == 0:
                        nc.vector.tensor_copy(dst, src)
                    else:
                        nc.scalar.copy(dst, src)

                def transposes(ih, qc0, nqc, tag):
                    tp = pst.tile(
                        [128, 512], BF16, tag="pst",
                        name=f"tp{img}_{tag}_{ih}",
                    )
                    for s in range(nqc):
                        qc = qc0 + s
                        nc.tensor.matmul(
                            tp[:, 128 * s : 128 * (s + 1)],
                            y_sb[:, ih, 128 * qc : 128 * (qc + 1)],
                            ident[:],
                            is_transpose=True,
                            start=(s == 0),
                            stop=(s == nqc - 1),
                        )
                    dst = yt_sb[:, qc0 : qc0 + nqc, 128 * ih : 128 * (ih + 1)]
                    tsrc = tp[:, 0 : 128 * nqc].rearrange("p (s w) -> p s w", s=nqc)
                    evac(dst, tsrc, ih)

                def p2mm(acc, qc, ih, jslice, start, stop):
                    nc.tensor.matmul(
                        acc,
                        yt_sb[:, qc, 128 * ih : 128 * (ih + 1)],
                        ttb[:, qc, jslice],
                        start=start,
                        stop=stop,
                    )

                if img < IMGS - 1:
                    # row-block chunked loads; 4 KB descriptors
                    nc.sync.dma_start(out=xt[:, 0:4], in_=xr[:, 0:4])
                    nc.sync.dma_start(out=xt[:, 4:8], in_=xr[:, 4:8])
                    for ch in range(2):
                        for ih in range(2):
                            yq = psy.tile(
                                [128, 512], F32, tag="psy",
                                name=f"psy{img}_{ch}_{ih}",
                            )
                            pcs = pcs_by_ih[ih]
                            for k, pc in enumerate(pcs):
                                p1mm(yq[:], pc, ih,
                                     xt[:, pc, 512 * ch : 512 * (ch + 1)],
                                     k == 0, k == len(pcs) - 1)
                            evac(y_sb[:, ih, 512 * ch : 512 * (ch + 1)], yq[:], ih)
                        for ih in range(2):
                            transposes(ih, 4 * ch, 4, f"c{ch}")
                    for ih in range(2):
                        acc = ps2.tile([128, HO], F32, tag="ps2",
                                       name=f"ps2_{img}_{ih}")
                        for qc in range(8):
                            p2mm(acc[:], qc, ih, slice(0, HO), qc == 0, qc == 7)
                        evac(z[:, ih, :], acc[:], ih)
                    nc.scalar.dma_start(out=out[:, img], in_=z[:])
                else:
                    # last image: ch0 (cols 0-511, f32r), then cols 512-1023
                    # in three row-block groups cast to bf16 in-flight
                    # (2 KB source descriptors throughout)
                    xb = xbpool.tile([128, 8, 512], BF16, tag="xb")
                    nc.sync.dma_start(out=xt[:, :, 0:512], in_=xr[:, :, 0:512])
                    nc.sync.dma_start(out=xt[:, 0:4, 512:1024],
                                        in_=xr[:, 0:4, 512:1024])
                    nc.sync.dma_start(out=xt[:, 4:6, 512:1024],
                                        in_=xr[:, 4:6, 512:1024])
                    nc.sync.dma_start(out=xt[:, 6:8, 512:1024],
                                        in_=xr[:, 6:8, 512:1024])

                    # ch0: classic pass 1 in bf16 (the f32r moving
                    # operand runs at 2 cycles/col, so casting ch0 on both
                    # copy engines in parallel then matmul-ing in bf16
                    # shortens the post-arrival serial chain by ~1 us) +
                    # transposes -> yt qc 0-3
                    xb0 = xbpool.tile([128, 8, 512], BF16, tag="xb0")
                    nc.vector.tensor_copy(xb0[:, 0:4], xt[:, 0:4, 0:512])
                    nc.scalar.copy(xb0[:, 4:8], xt[:, 4:8, 0:512])
                    for ih in range(2):
                        yq = psy.tile([128, 512], F32, tag="psy",
                                      name=f"psyL_{ih}")
                        pcs = pcs_by_ih[ih]
                        for k, pc in enumerate(pcs):
                            nc.tensor.matmul(
                                yq[:],
                                ttb[:, pc, 128 * ih : 128 * (ih + 1)],
                                xb0[:, pc, :],
                                start=(k == 0),
                                stop=(k == len(pcs) - 1),
                            )
                        evac(y_sb[:, ih, 0:512], yq[:], ih)
                    for ih in range(2):
                        transposes(ih, 0, 4, "L")

                    # ch1 row-groups are cast to bf16 as they arrive (the
                    # in-flight SWDGE cast drains erratically, so the x
                    # stream stays f32r).  DVE/ACT are strict FIFO, so each
                    # cast is emitted at its pipeline position: a cast that
                    # waits on a late chunk must not precede early evacs.

                    # cols 512-1023 via transposed pass 1: Yt[qc 4-7]
                    # accumulated with bf16 x row-blocks as stationary (FWL)
                    # against banded Tt windows, into three PSUM region
                    # tiles split at the pc4/pc6 window starts so each
                    # region's accumulation closes as soon as its last
                    # contributing row-block arrives (A <- pc3, B <- pc5,
                    # C <- pc7) and is evacuated immediately.
                    SA, SB = wins[4][0], wins[6][0]
                    ytqA = psy.tile([128, 4, SA], F32, tag="psy", name="ytqA")
                    ytqB = psy.tile([128, 4, SB - SA], F32, tag="psy",
                                    name="ytqB")
                    ytqC = psy.tile([128, 4, HO - SB], F32, tag="psy",
                                    name="ytqC")
                    regions = [(0, SA, ytqA), (SA, SB, ytqB), (SB, HO, ytqC)]

                    sched = []
                    for pc in range(8):
                        a, b = wins[pc]
                        for qc in (4, 5, 6, 7):
                            for ti, (ra, rb, _t) in enumerate(regions):
                                sa, sb = max(a, ra), min(b, rb)
                                if sa < sb:
                                    sched.append((pc, qc, ti, sa, sb))
                    first_w = {}
                    last_w = {}
                    for w in sched:
                        first_w.setdefault(w[2], w)
                        last_w[w[2]] = w

                    def p1t(pcg):
                        for pc in pcg:
                            a, b = wins[pc]
                            for qc in (4, 5, 6, 7):
                                for ti, (ra, rb, t) in enumerate(regions):
                                    sa, sb = max(a, ra), min(b, rb)
                                    if sa >= sb:
                                        continue
                                    w = (pc, qc, ti, sa, sb)
                                    nc.tensor.matmul(
                                        t[:, qc - 4, sa - ra : sb - ra],
                                        xb[:, pc,
                                           128 * (qc - 4) : 128 * (qc - 3)],
                                        ttb[:, pc, sa:sb],
                                        start=(w == first_w[ti]),
                                        stop=(w == last_w[ti]),
                                    )

                    # zA: narrow qc 0-3 block for cols 0-125, computed in
                    # the PE idle gap after ch0 (before the ch1 chunks land)
                    # and stored early
                    for ih in range(2):
                        acc = ps2.tile([128, ZCUT], F32, tag="ps2",
                                       name=f"ps2A_{ih}")
                        for qc in range(4):
                            p2mm(acc[:], qc, ih, slice(0, ZCUT),
                                 qc == 0, qc == 3)
                        evac(z[:, ih, 0:ZCUT], acc[:], ih)
                    nc.sync.dma_start(out=out[:, img, :, 0:ZCUT],
                                       in_=z[:, :, 0:ZCUT])

                    # z cols 126-255 accumulate per ih in separate banks;
                    # the qc3 contribution (from ch0) starts each bank early
                    zb0 = ps2.tile([128, HO - ZCUT], F32, tag="ps2",
                                   name="zb0")
                    zb1 = ps2.tile([128, HO - ZCUT], F32, tag="ps2",
                                   name="zb1")
                    nc.tensor.matmul(zb0[:], yt_sb[:, 3, 0:128],
                                     ttb[:, 3, ZCUT:HO],
                                     start=True, stop=False)
                    nc.tensor.matmul(zb1[:], yt_sb[:, 3, 128:256],
                                     ttb[:, 3, ZCUT:HO],
                                     start=True, stop=False)

                    nc.vector.tensor_copy(xb[:, 0:4], xt[:, 0:4, 512:1024])
                    p1t((0, 1, 2, 3))
                    nc.vector.tensor_copy(yt_sb[:, 4:8, 0:SA], ytqA[:])
                    nc.scalar.copy(xb[:, 4:6], xt[:, 4:6, 512:1024])
                    p1t((4, 5))
                    nc.scalar.copy(yt_sb[:, 4:8, SA:SB], ytqB[:])

                    # z[:, 0, 126:] needs Yt cols 0:128 (regions A+B):
                    # computed and stored while pc6/pc7 still stream
                    for qc in range(4, 8):
                        nc.tensor.matmul(zb0[:], yt_sb[:, qc, 0:128],
                                         ttb[:, qc, ZCUT:HO],
                                         start=False, stop=(qc == 7))
                    nc.vector.tensor_copy(z[:, 0, ZCUT:HO], zb0[:])
                    nc.sync.dma_start(out=out[:, img, 0, ZCUT:HO],
                                       in_=z[:, 0, ZCUT:HO])

                    nc.vector.tensor_copy(xb[:, 6:7], xt[:, 6:7, 512:1024])
                    nc.scalar.copy(xb[:, 7:8], xt[:, 7:8, 512:1024])
                    p1t((6, 7))
                    nc.scalar.copy(yt_sb[:, 4:8, SB:HO], ytqC[:])
                    for qc in range(4, 8):
                        nc.tensor.matmul(zb1[:], yt_sb[:, qc, 128:256],
                                         ttb[:, qc, ZCUT:HO],
                                         start=False, stop=(qc == 7))
                    nc.vector.tensor_copy(z[:, 1, ZCUT:HO], zb1[:])
                    nc.sync.dma_start(out=out[:, img, 1, ZCUT:HO],
                                       in_=z[:, 1, ZCUT:HO])
    nc.compile()
    return nc


_GRAPH = None


def _get_graph():
    global _GRAPH
    if _GRAPH is None:
        _GRAPH = _build_graph()
    return _GRAPH


def run(x, **spmd_kwargs):
    x = np.ascontiguousarray(np.asarray(x, dtype=np.float32))
    assert x.shape == (B, C, H, W)
    nc = _get_graph()
    Tt = build_T().T  # [1024, 256] f32
    tb_host = np.ascontiguousarray(
        Tt.reshape(8, 128, HO).transpose(1, 0, 2)
    ).astype(ml_dtypes.bfloat16)
    eye_host = np.eye(128, dtype=ml_dtypes.bfloat16)
    per_core = B // N_CORES
    in_maps = [
        {
            "x": x[i * per_core : (i + 1) * per_core].reshape(IMGS, H, W),
            "tb": tb_host,
            "eye": eye_host,
        }
        for i in range(N_CORES)
    ]
    res = run_bass_kernel_spmd(nc, in_maps, core_ids=list(range(N_CORES)), **spmd_kwargs)
    outs = []
    for r in res.results:
        o = r["out"].transpose(1, 2, 0, 3).reshape(IMGS, 2 * 128, HO)
        outs.append(o.reshape(per_core, C, HO, HO))
    return np.concatenate(outs, axis=0), res


def kernel(x):
    out, _ = run(x)
    return out



# revision 33
# speedup vs baseline: 1.1535x; 1.0190x over previous
"""Antialiased bicubic 4x downscale (blur -> bicubic/2, twice) on 8 TRN2 cores.

The whole chain is linear and separable: every stage is M_H (x) M_W acting on
the H/W axes, so the composition collapses to a single 1024->256 banded matrix
T applied on both sides: out = T @ X @ T^T per (batch, channel) image.

Sharding: pure data parallel - batch 16 -> 2 per core, 6 images/core.

The kernel is HBM-bound: 24 MiB of x per core streams at line rate (~360
GB/s, the 8-core fair share of the chip HBM ceiling) on the sync HWDGE ring
-- HWDGE starts ~1.3 us earlier than SWDGE and is immune to the DVE 2-port
perf-mode lockout that starves SWDGE descriptor generation.  Mid-stream z
stores ride the scalar HWDGE ring; the last image's stores go back to the
(by then idle) sync engine so store issues never sit in ACT's evac FIFO.
Descriptors are kept >= 2 KB, and regions being DMA-written are never read
concurrently (both measurably collapse the drain rate).  All compute
(pass 1 f32r matmuls exploiting T's band sparsity, PE transposes, bf16
pass 2) hides under the stream; work after the last byte is minimized:

  The last image arrives as ch0 (cols 0-511, cast to bf16 on both copy
  engines in parallel and processed classically -- bf16 matmuls halve the
  post-arrival serial chain vs f32r),
  then cols 512-1023 in three row-block groups, cast to bf16 on-chip.
  Those columns take a transposed pass 1: Yt[qc 4-7] accumulate with the
  bf16 x row-blocks as stationary against banded Tt windows, into three
  PSUM region tiles split at the pc4/pc6 window starts (cols 126/190) so
  each region evacuates as soon as its last contributing row-block is in.
  z is split at column 126: z[:, :, 0:126] (qc 0-3 only) stores early;
  z[:, 0, 126:256] (needs Yt cols 0:128 = regions A+B) computes and
  stores while the final chunks stream; after the last byte only the
  pc6/7 windowed matmuls, one region evac, 4 pass-2 matmuls, one evac
  and one 130-column store remain.
"""

import numpy as np
import ml_dtypes

import concourse.bacc as bacc
import concourse.mybir as mybir
import concourse.tile as tile
from concourse.bass_utils import run_bass_kernel_spmd

SIGMA = 0.66
BICUBIC_W = np.array([-0.09375, 0.59375, 0.59375, -0.09375], dtype=np.float64)

N_CORES = 8
B, C, H, W = 16, 3, 1024, 1024
HO = H // 4
IMGS = (B // N_CORES) * C  # 6 images per core

F32 = mybir.dt.float32
F32R = mybir.dt.float32r
BF16 = mybir.dt.bfloat16

# z column split for the last image: z[:, 0:ZCUT] depends only on qc 0-3
# (x cols 0-511) and is stored early; z[:, ZCUT:] additionally gets the
# qc 4-7 contributions added after the stream ends.
ZCUT = 126


def _gauss_matrix(n):
    x = np.arange(3, dtype=np.float32) - np.float32(1.0)
    k = np.exp(np.float32(-0.5) * (x / np.float32(SIGMA)) ** 2)
    k = (k / k.sum()).astype(np.float64)
    G = np.zeros((n, n))
    for t in range(3):
        G += k[t] * np.eye(n, n, t - 1)
    return G


def _down_matrix(n):
    # out[i] = sum_t w[t] * x[clamp(2i + t - 1, 0, n-1)]
    m = n // 2
    D = np.zeros((m, n))
    for i in range(m):
        for t in range(4):
            j = min(max(2 * i + t - 1, 0), n - 1)
            D[i, j] += BICUBIC_W[t]
    return D


def build_T():
    T = _down_matrix(H // 2) @ _gauss_matrix(H // 2) @ _down_matrix(H) @ _gauss_matrix(H)
    return T.astype(np.float32)  # [256, 1024]


def _pass1_pieces(Tt):
    """(pc, ih) pairs where Tt[128pc:128pc+128, 128ih:128ih+128] is nonzero."""
    pieces = []
    for ih in range(2):
        for pc in range(8):
            if np.any(Tt[128 * pc : 128 * (pc + 1), 128 * ih : 128 * (ih + 1)]):
                pieces.append((pc, ih))
    return pieces


def _pc_windows(Tt):
    """Per row-block pc, the [a, b) span of nonzero columns of Tt."""
    wins = []
    for pc in range(8):
        nz = np.nonzero(np.any(Tt[128 * pc : 128 * (pc + 1), :] != 0, axis=0))[0]
        wins.append((int(nz.min()), int(nz.max()) + 1))
    return wins


def _build_graph():
    Tt = build_T().T  # [1024, 256]
    pieces = _pass1_pieces(Tt)
    pcs_by_ih = [[pc for (pc, ih2) in pieces if ih2 == ih] for ih in range(2)]
    wins = _pc_windows(Tt)

    nc = bacc.Bacc("TRN2", target_bir_lowering=False, debug=False)
    x = nc.dram_tensor("x", [IMGS, H, W], F32R, kind="ExternalInput").ap()
    # tb is host-prearranged to the SBUF layout: tb[p, c, n] = Tt[128c+p, n]
    tb = nc.dram_tensor("tb", [128, 8, HO], BF16, kind="ExternalInput").ap()
    eye = nc.dram_tensor("eye", [128, 128], BF16, kind="ExternalInput").ap()
    # out in SBUF layout [p, img, c, j] = Z[img, 128c+p, j]; host unscrambles
    out = nc.dram_tensor("out", [128, IMGS, 2, HO], F32, kind="ExternalOutput").ap()

    with tile.TileContext(nc) as tc:
        with (
            tc.tile_pool(name="const", bufs=1) as cpool,
            tc.tile_pool(name="xin", bufs=4) as xpool,
            tc.tile_pool(name="xbin", bufs=1) as xbpool,
            tc.tile_pool(name="ysb", bufs=2) as ypool,
            tc.tile_pool(name="ytsb", bufs=2) as ytpool,
            tc.tile_pool(name="zout", bufs=2) as zpool,
            tc.tile_pool(name="psy", bufs=4, space="PSUM") as psy,
            tc.tile_pool(name="pst", bufs=2, space="PSUM") as pst,
            tc.tile_pool(name="ps2", bufs=2, space="PSUM") as ps2,
        ):
            ttb = cpool.tile([128, 8, HO], BF16, tag="ttb")
            nc.scalar.dma_start(out=ttb[:], in_=tb)
            ident = cpool.tile([128, 128], BF16, tag="ident")
            nc.scalar.dma_start(out=ident[:], in_=eye)
            # f32r copy of Tt for pass 1 stationary, cast on-chip
            tt = cpool.tile([128, 8, HO], F32R, tag="tt")
            nc.vector.tensor_copy(tt[:], ttb[:])

            def p1mm(yq, pc, ih, xap, start, stop):
                nc.tensor.matmul(
                    yq,
                    tt[:, pc, 128 * ih : 128 * (ih + 1)],
                    xap,
                    start=start,
                    stop=stop,
                )

            for img in range(IMGS):
                xt = xpool.tile([128, 8, W], F32R, tag="xt", name=f"xt{img}")
                xr = x[img].rearrange("(c p) w -> p c w", p=128)

                y_sb = ypool.tile([128, 2, W], BF16)
                yt_sb = ytpool.tile([128, 8, HO], BF16)
                z = zpool.tile([128, 2, HO], F32, tag="zout", name=f"z{img}")

                def evac(dst, src, ih):
                    if ih == 0:
                        nc.vector.tensor_copy(dst, src)
                    else:
                        nc.scalar.copy(dst, src)

                def transposes(ih, qc0, nqc, tag):
                    tp = pst.tile(
                        [128, 512], BF16, tag="pst",
                        name=f"tp{img}_{tag}_{ih}",
                    )
                    for s in range(nqc):
                        qc = qc0 + s
                        nc.tensor.matmul(
                            tp[:, 128 * s : 128 * (s + 1)],
                            y_sb[:, ih, 128 * qc : 128 * (qc + 1)],
                            ident[:],
                            is_transpose=True,
                            start=(s == 0),
                            stop=(s == nqc - 1),
                        )
                    dst = yt_sb[:, qc0 : qc0 + nqc, 128 * ih : 128 * (ih + 1)]
                    tsrc = tp[:, 0 : 128 * nqc].rearrange("p (s w) -> p s w", s=nqc)
                    evac(dst, tsrc, ih)

                def p2mm(acc, qc, ih, jslice, start, stop):
                    nc.tensor.matmul(
                        acc,
                        yt_sb[:, qc, 128 * ih : 128 * (ih + 1)],
                        ttb[:, qc, jslice],
                        start=start,
                        stop=stop,
                    )

                if img < IMGS - 1:
                    # row-block chunked loads; 4 KB descriptors
                    nc.sync.dma_start(out=xt[:, 0:4], in_=xr[:, 0:4])
                    nc.sync.dma_start(out=xt[:, 4:8], in_=xr[:, 4:8])
                    for ch in range(2):
                        for ih in range(2):
                            yq = psy.tile(
                                [128, 512], F32, tag="psy",
                                name=f"psy{img}_{ch}_{ih}",
                            )
                            pcs = pcs_by_ih[ih]
                            for k, pc in enumerate(pcs):
                                p1mm(yq[:], pc, ih,
                                     xt[:, pc, 512 * ch : 512 * (ch + 1)],
                                     k == 0, k == len(pcs) - 1)
                            evac(y_sb[:, ih, 512 * ch : 512 * (ch + 1)], yq[:], ih)
                        for ih in range(2):
                            transposes(ih, 4 * ch, 4, f"c{ch}")
                    for ih in range(2):
                        acc = ps2.tile([128, HO], F32, tag="ps2",
                                       name=f"ps2_{img}_{ih}")
                        for qc in range(8):
                            p2mm(acc[:], qc, ih, slice(0, HO), qc == 0, qc == 7)
                        evac(z[:, ih, :], acc[:], ih)
                    nc.scalar.dma_start(out=out[:, img], in_=z[:])
                else:
                    # last image: ch0 (cols 0-511, f32r), then cols 512-1023
                    # in three row-block groups cast to bf16 in-flight
                    # (2 KB source descriptors throughout)
                    xb = xbpool.tile([128, 8, 512], BF16, tag="xb")
                    nc.sync.dma_start(out=xt[:, :, 0:512], in_=xr[:, :, 0:512])
                    nc.sync.dma_start(out=xt[:, 0:4, 512:1024],
                                        in_=xr[:, 0:4, 512:1024])
                    nc.sync.dma_start(out=xt[:, 4:6, 512:1024],
                                        in_=xr[:, 4:6, 512:1024])
                    nc.sync.dma_start(out=xt[:, 6:8, 512:1024],
                                        in_=xr[:, 6:8, 512:1024])

                    # ch0: classic pass 1 in bf16 (the f32r moving
                    # operand runs at 2 cycles/col, so casting ch0 on both
                    # copy engines in parallel then matmul-ing in bf16
                    # shortens the post-arrival serial chain by ~1 us) +
                    # transposes -> yt qc 0-3
                    xb0 = xbpool.tile([128, 8, 512], BF16, tag="xb0")
                    nc.vector.tensor_copy(xb0[:, 0:4], xt[:, 0:4, 0:512])
                    nc.scalar.copy(xb0[:, 4:8], xt[:, 4:8, 0:512])
                    for ih in range(2):
                        yq = psy.tile([128, 512], F32, tag="psy",
                                      name=f"psyL_{ih}")
                        pcs = pcs_by_ih[ih]
                        for k, pc in enumerate(pcs):
                            nc.tensor.matmul(
                                yq[:],
                                ttb[:, pc, 128 * ih : 128 * (ih + 1)],
                                xb0[:, pc, :],
                                start=(k == 0),
                                stop=(k == len(pcs) - 1),
                            )
                        evac(y_sb[:, ih, 0:512], yq[:], ih)
                    for ih in range(2):
                        transposes(ih, 0, 4, "L")

                    # ch1 row-groups are cast to bf16 as they arrive (the
                    # in-flight SWDGE cast drains erratically, so the x
                    # stream stays f32r).  DVE/ACT are strict FIFO, so each
                    # cast is emitted at its pipeline position: a cast that
                    # waits on a late chunk must not precede early evacs.

                    # cols 512-1023 via transposed pass 1: Yt[qc 4-7]
                    # accumulated with bf16 x row-blocks as stationary (FWL)
                    # against banded Tt windows, into three PSUM region
                    # tiles split at the pc4/pc6 window starts so each
                    # region's accumulation closes as soon as its last
                    # contributing row-block arrives (A <- pc3, B <- pc5,
                    # C <- pc7) and is evacuated immediately.
                    SA, SB = wins[4][0], wins[6][0]
                    ytqA = psy.tile([128, 4, SA], F32, tag="psy", name="ytqA")
                    ytqB = psy.tile([128, 4, SB - SA], F32, tag="psy",
                                    name="ytqB")
                    ytqC = psy.tile([128, 4, HO - SB], F32, tag="psy",
                                    name="ytqC")
                    regions = [(0, SA, ytqA), (SA, SB, ytqB), (SB, HO, ytqC)]

                    sched = []
                    for pc in range(8):
                        a, b = wins[pc]
                        for qc in (4, 5, 6, 7):
                            for ti, (ra, rb, _t) in enumerate(regions):
                                sa, sb = max(a, ra), min(b, rb)
                                if sa < sb:
                                    sched.append((pc, qc, ti, sa, sb))
                    first_w = {}
                    last_w = {}
                    for w in sched:
                        first_w.setdefault(w[2], w)
                        last_w[w[2]] = w

                    def p1t(pcg):
                        for pc in pcg:
                            a, b = wins[pc]
                            for qc in (4, 5, 6, 7):
                                for ti, (ra, rb, t) in enumerate(regions):
                                    sa, sb = max(a, ra), min(b, rb)
                                    if sa >= sb:
                                        continue
                                    w = (pc, qc, ti, sa, sb)
                                    nc.tensor.matmul(
                                        t[:, qc - 4, sa - ra : sb - ra],
                                        xb[:, pc,
                                           128 * (qc - 4) : 128 * (qc - 3)],
                                        ttb[:, pc, sa:sb],
                                        start=(w == first_w[ti]),
                                        stop=(w == last_w[ti]),
                                    )

                    # zA: narrow qc 0-3 block for cols 0-125, computed in
                    # the PE idle gap after ch0 (before the ch1 chunks land)
                    # and stored early
                    for ih in range(2):
                        acc = ps2.tile([128, ZCUT], F32, tag="ps2",
                                       name=f"ps2A_{ih}")
                        for qc in range(4):
                            p2mm(acc[:], qc, ih, slice(0, ZCUT),
                                 qc == 0, qc == 3)
                        evac(z[:, ih, 0:ZCUT], acc[:], ih)
                    nc.sync.dma_start(out=out[:, img, :, 0:ZCUT],
                                       in_=z[:, :, 0:ZCUT])

                    # z cols 126-255 accumulate per ih in separate banks;
                    # the qc3 contribution (from ch0) starts each bank early
                    zb0 = ps2.tile([128, HO - ZCUT], F32, tag="ps2",
                                   name="zb0")
                    zb1 = ps2.tile([128, HO - ZCUT], F32, tag="ps2",
                                   name="zb1")
                    nc.tensor.matmul(zb0[:], yt_sb[:, 3, 0:128],
                                     ttb[:, 3, ZCUT:HO],
                                     start=True, stop=False)
                    nc.tensor.matmul(zb1[:], yt_sb[:, 3, 128:256],
                                     ttb[:, 3, ZCUT:HO],
                                     start=True, stop=False)

                    nc.vector.tensor_copy(xb[:, 0:4], xt[:, 0:4, 512:1024])
                    p1t((0, 1, 2, 3))
                    nc.vector.tensor_copy(yt_sb[:, 4:8, 0:SA], ytqA[:])
                    nc.scalar.copy(xb[:, 4:6], xt[:, 4:6, 512:1024])
                    p1t((4, 5))
                    nc.scalar.copy(yt_sb[:, 4:8, SA:SB], ytqB[:])

                    # z[:, 0, 126:] needs Yt cols 0:128 (regions A+B):
                    # computed and stored while pc6/pc7 still stream
                    for qc in range(4, 8):
                        nc.tensor.matmul(zb0[:], yt_sb[:, qc, 0:128],
                                         ttb[:, qc, ZCUT:HO],
                                         start=False, stop=(qc == 7))
                    nc.vector.tensor_copy(z[:, 0, ZCUT:HO], zb0[:])
                    nc.sync.dma_start(out=out[:, img, 0, ZCUT:HO],
                                       in_=z[:, 0, ZCUT:HO])

                    nc.vector.tensor_copy(xb[:, 6:7], xt[:, 6:7, 512:1024])
                    nc.scalar.copy(xb[:, 7:8], xt[:, 7:8, 512:1024])
                    p1t((6, 7))
                    nc.scalar.copy(yt_sb[:, 4:8, SB:HO], ytqC[:])
                    for qc in range(4, 8):
                        nc.tensor.matmul(zb1[:], yt_sb[:, qc, 128:256],
                                         ttb[:, qc, ZCUT:HO],
                                         start=False, stop=(qc == 7))
                    nc.vector.tensor_copy(z[:, 1, ZCUT:HO], zb1[:])
                    nc.sync.dma_start(out=out[:, img, 1, ZCUT:HO],
                                       in_=z[:, 1, ZCUT:HO])
    nc.compile()
    return nc


_GRAPH = None


def _get_graph():
    global _GRAPH
    if _GRAPH is None:
        _GRAPH = _build_graph()
    return _GRAPH


def run(x, **spmd_kwargs):
    x = np.ascontiguousarray(np.asarray(x, dtype=np.float32))
    assert x.shape == (B, C, H, W)
    nc = _get_graph()
    Tt = build_T().T  # [1024, 256] f32
    tb_host = np.ascontiguousarray(
        Tt.reshape(8, 128, HO).transpose(1, 0, 2)
    ).astype(ml_dtypes.bfloat16)
    eye_host = np.eye(128, dtype=ml_dtypes.bfloat16)
    per_core = B // N_CORES
    in_maps = [
        {
            "x": x[i * per_core : (i + 1) * per_core].reshape(IMGS, H, W),
            "tb": tb_host,
            "eye": eye_host,
        }
        for i in range(N_CORES)
    ]
    res = run_bass_kernel_spmd(nc, in_maps, core_ids=list(range(N_CORES)), **spmd_kwargs)
    outs = []
    for r in res.results:
        o = r["out"].transpose(1, 2, 0, 3).reshape(IMGS, 2 * 128, HO)
        outs.append(o.reshape(per_core, C, HO, HO))
    return np.concatenate(outs, axis=0), res


def kernel(x):
    out, _ = run(x)
    return out

